# revision 13
# baseline (speedup 1.0000x reference)
"""Attention pooling (segment softmax + weighted segment-mean) on 8 Trainium2 cores.

Reference computation (per full input):
    logits = leaky_relu(feature @ a, 0.2)                    # [N]
    att    = segment_softmax(logits, batch)                  # [N]
    out    = segment_sum(att[:, None] * feature) / counts    # [1024, 256]

Structure (all on-device data bf16, fp32 accumulation):
  * Host pre-multiplies `a` into the features: G = feature * a^T. The
    logit matvec degenerates to a row-sum of G, and the weighted segment
    sums come out scaled by a_h, which the host divides back out (errors
    scale with a_h, so no precision is lost).
  * Sorted batch ids -> 8 blocks of 128 contiguous segments (1/core),
    4 groups of 32 segments per core, each group padded to 13 supertiles
    of 512 nodes (4 subtiles x 128).  Supertiles are processed in
    batches of 4 (16 subtiles) so every engine op covers 16 subtiles.
  * DMA row per subtile: [256 G | 1.0 | pad3 | 32 one-hot mask] = 292
    bf16 = 584B; a batch line is 16*584 = 9344B contiguous per
    partition, split across the two HWDGE rings (4672B descriptors,
    ~370 GB/s measured).  The 1.0 feeds the denominator column; the
    one-hot mask (vs the group-relative segment id) feeds W.
  * Per batch: DVE folds G 256->128->64->32 (bf16 2x mode) + one
    tensor_reduce -> z [128,16]; ACT Prelu(0.2) + Exp -> ex; DVE builds
    W = mask * ex (one op); PE accumulates [sums | denom] += W.T @ [G|1]
    into the group's 32 PSUM rows (13x4 subtile chain per group).
Counts and the final (sums / denom / counts / a) normalization are
O(segments) and done on host.
"""

from contextlib import ExitStack

import numpy as np

import concourse.bacc as bacc
import concourse.tile as tile
from concourse import mybir
from concourse.bass_utils import run_bass_kernel_spmd

N_CORES = 8
P = 128                 # partitions / nodes per subtile
H = 256                 # hidden
NSEG = 1024
SEG_PER_CORE = NSEG // N_CORES   # 128
K = 4                   # subtiles per supertile
GSEG = 32               # segments per group
NGROUP = SEG_PER_CORE // GSEG    # 4 groups per core
SUP_PER_GROUP = 13      # supertiles per group (6656 nodes >= max group ~6415)
NSUP = NGROUP * SUP_PER_GROUP    # 52 supertiles
GROUP_CAP = SUP_PER_GROUP * K * P   # 6656 nodes per group
NP = NSUP * K * P       # 26624 padded nodes per core
ROW = H + 2 + GSEG      # 290: [256 G | 1.0 | 1 pad | 32 mask]
MASK0 = H + 2           # mask column offset (258 elems = 516B, 4B-aligned)
BATCH = 4               # supertiles per batch
NB = NSUP // BATCH      # 13 batches
C = K * BATCH           # 16 subtiles per batch
CA = 2                  # subtiles per batch reduced on ACT instead of DVE
CD = C - CA             # subtiles per batch reduced on the DVE fold cascade
NEG_SLOPE = 0.2

_G, _OUT = "gfeat", "out"
F32 = mybir.dt.float32
BF16 = mybir.dt.bfloat16
ALU = mybir.AluOpType


def _build_program():
    nc = bacc.Bacc("TRN2", target_bir_lowering=False, debug=False)
    g_d = nc.dram_tensor(_G, [P, NB * C * ROW], BF16, kind="ExternalInput").ap()
    out_d = nc.dram_tensor(_OUT, [P, H + 1], F32, kind="ExternalOutput").ap()
    g_r = g_d.rearrange("p (b c r) -> p b c r", b=NB, c=C)

    with tile.TileContext(nc) as tc, ExitStack() as ctx:
        gpool = ctx.enter_context(tc.tile_pool(name="g", bufs=6))
        fpool = ctx.enter_context(tc.tile_pool(name="f", bufs=2))
        spool = ctx.enter_context(tc.tile_pool(name="s", bufs=1))
        zpool = ctx.enter_context(tc.tile_pool(name="z", bufs=3))
        wpool = ctx.enter_context(tc.tile_pool(name="w", bufs=2))
        opool = ctx.enter_context(tc.tile_pool(name="o", bufs=1))
        psum = ctx.enter_context(tc.tile_pool(name="psum", bufs=1, space="PSUM"))

        acc = psum.tile([P, H + 1], F32, tag="acc")
        ascr = spool.tile([P, H], BF16, tag="ascr")  # ACT accum scratch out

        def z_and_ex(b, Gb):
            """Reduce: DVE fold cascade (CD subtiles) + ACT accum (CA);
            then ACT prelu+exp -> exb."""
            zb = zpool.tile([P, C], F32, name="zb")
            f1 = fpool.tile([P, CD, 128], BF16, name="f1")
            nc.vector.tensor_tensor(out=f1, in0=Gb[:, 0:CD, 0:128],
                                    in1=Gb[:, 0:CD, 128:256], op=ALU.add)
            f2 = fpool.tile([P, CD, 64], BF16, name="f2")
            nc.vector.tensor_tensor(out=f2, in0=f1[:, :, 0:64],
                                    in1=f1[:, :, 64:128], op=ALU.add)
            f3 = fpool.tile([P, CD, 32], BF16, name="f3")
            nc.vector.tensor_tensor(out=f3, in0=f2[:, :, 0:32],
                                    in1=f2[:, :, 32:64], op=ALU.add)
            nc.vector.tensor_reduce(out=zb[:, 0:CD], in_=f3,
                                    axis=mybir.AxisListType.X, op=ALU.add)
            for c in range(CD, C):
                nc.scalar.activation(ascr, Gb[:, c, 0:H],
                                     mybir.ActivationFunctionType.Copy,
                                     accum_out=zb[:, c:c + 1])
            lb = zpool.tile([P, C], F32, name="lb")
            nc.scalar.activation(lb, zb, mybir.ActivationFunctionType.Prelu,
                                 alpha=NEG_SLOPE)
            exb = zpool.tile([P, C], F32, name="exb")
            nc.scalar.activation(exb, lb, mybir.ActivationFunctionType.Exp)
            return exb

        def w_and_matmul(b, Gb, exb):
            W16 = wpool.tile([P, C, GSEG], BF16, name="W16")
            nc.vector.tensor_tensor(
                out=W16, in0=Gb[:, :, MASK0:ROW],
                in1=exb[:, :, None].broadcast_to([P, C, GSEG]),
                op=ALU.mult)
            for c in range(C):
                s = b * BATCH + c // K
                g = s // SUP_PER_GROUP
                j = s % SUP_PER_GROUP
                k = c % K
                nc.tensor.matmul(acc[g * GSEG:(g + 1) * GSEG, :],
                                 lhsT=W16[:, c, :], rhs=Gb[:, c, 0:H + 1],
                                 start=(j == 0 and k == 0),
                                 stop=(j == SUP_PER_GROUP - 1 and k == K - 1),
                                 tile_position=(0, g * GSEG))

        out_sb = opool.tile([P, H + 1], F32, tag="out_sb")

        def emit_group_out(g):
            r0, r1 = g * GSEG, (g + 1) * GSEG
            nc.scalar.copy(out_sb[r0:r1, :], acc[r0:r1, :])
            nc.scalar.dma_start(out_d[r0:r1, :], out_sb[r0:r1, :])

        # group g's accumulation chain closes during batch (13g+12)//4;
        # emit its output 3 batches later so the stop-matmul has retired
        # and the in-order ACT queue never stalls on it.
        out_at = {((SUP_PER_GROUP * (g + 1) - 1) // BATCH) + 3: g
                  for g in range(NGROUP)}

        pending = None          # (b, Gb, exb) awaiting W+matmul
        for b in range(NB):
            Gb = gpool.tile([P, C, ROW], BF16, name="Gb")
            # gpsimd (SWDGE) ring finishes its preamble first; use it for
            # the first batches so compute starts ~5us earlier.
            ring = nc.gpsimd if b < 2 else nc.sync
            ring.dma_start(Gb, g_r[:, b])
            exb = z_and_ex(b, Gb)
            if pending is not None:
                w_and_matmul(*pending)
            if b in out_at:
                emit_group_out(out_at[b])
            pending = (b, Gb, exb)
        w_and_matmul(*pending)
        emit_group_out(NGROUP - 1)

    nc.compile()
    return nc


def _to_bf16(x):
    return np.asarray(x, dtype=np.float32).astype(mybir.dt.np(BF16))


def kernel(feature, a, batch, _trace=False):
    feature = np.asarray(feature, dtype=np.float32)
    a = np.asarray(a, dtype=np.float32)
    batch = np.asarray(batch)
    n = feature.shape[0]
    assert feature.shape == (n, H) and batch.shape == (n,)

    avec = a.reshape(-1)                      # [256]
    gfull = feature * avec[None, :]           # G = F * a  (fp32, exact mult)

    gbounds = np.searchsorted(batch, np.arange(0, NSEG + 1, GSEG))

    in_maps = []
    for c in range(N_CORES):
        g_c = np.zeros((NP, ROW), dtype=np.float32)
        g_c[:, H] = 1.0                       # denominator ones column
        for g in range(NGROUP):
            gi = c * NGROUP + g
            s, e = int(gbounds[gi]), int(gbounds[gi + 1])
            cnt = e - s
            assert cnt <= GROUP_CAP, (
                f"core {c} group {g} has {cnt} nodes > capacity {GROUP_CAP}")
            base = g * GROUP_CAP
            g_c[base:base + cnt, 0:H] = gfull[s:e]
            seg_rel = batch[s:e].astype(np.int64) - (c * SEG_PER_CORE + g * GSEG)
            g_c[np.arange(base, base + cnt), MASK0 + seg_rel] = 1.0  # one-hot
        # [NP, ROW] -> [NSUP, K, P, ROW] -> [P, (NSUP K ROW)]
        g_t = _to_bf16(
            g_c.reshape(NSUP, K, P, ROW).transpose(2, 0, 1, 3).reshape(P, -1))
        in_maps.append({_G: np.ascontiguousarray(g_t)})

    nc = _build_program()
    res = run_bass_kernel_spmd(nc, in_maps, core_ids=list(range(N_CORES)),
                               trace=_trace)

    counts = np.bincount(batch.astype(np.int64), minlength=NSEG).astype(np.float32)
    counts = np.maximum(counts, 1.0)
    safe_a = np.where(np.abs(avec) > 1e-30, avec, 1e-30)  # [256]
    out = np.zeros((NSEG, H), dtype=np.float32)
    for c in range(N_CORES):
        blk = res.results[c][_OUT]          # [128, 257]
        sums, denom = blk[:, :H], blk[:, H]
        seg0 = c * SEG_PER_CORE
        safe = np.maximum(denom, 1e-30)[:, None]
        out[seg0:seg0 + SEG_PER_CORE] = np.where(
            denom[:, None] > 0.0,
            sums / safe / counts[seg0:seg0 + SEG_PER_CORE, None] / safe_a[None, :],
            0.0,
        )
    if _trace:
        kernel.last_results = res
    return out


# revision 14
# speedup vs baseline: 1.1713x; 1.1713x over previous
"""Attention pooling (segment softmax + weighted segment-mean) on 8 Trainium2 cores.

Reference computation (per full input):
    logits = leaky_relu(feature @ a, 0.2)                    # [N]
    att    = segment_softmax(logits, batch)                  # [N]
    out    = segment_sum(att[:, None] * feature) / counts    # [1024, 256]

Structure (all on-device data bf16, fp32 accumulation):
  * Host pre-multiplies `a` into the features: G = feature * a^T. The
    logit matvec degenerates to a row-sum of G, and the weighted segment
    sums come out scaled by a_h, which the host divides back out (errors
    scale with a_h, so no precision is lost).
  * Sorted batch ids -> 8 blocks of 128 contiguous segments (1/core),
    4 groups of 32 segments per core, each group padded to 13 supertiles
    of 512 nodes (4 subtiles x 128).  Supertiles are processed in
    batches of 4 (16 subtiles) so every engine op covers 16 subtiles.
  * DMA row per subtile: [256 G | 1.0 | pad3 | 32 one-hot mask] = 292
    bf16 = 584B; a batch line is 16*584 = 9344B contiguous per
    partition, split across the two HWDGE rings (4672B descriptors,
    ~370 GB/s measured).  The 1.0 feeds the denominator column; the
    one-hot mask (vs the group-relative segment id) feeds W.
  * Per batch: DVE folds G 256->128->64->32 (bf16 2x mode) + one
    tensor_reduce -> z [128,16]; ACT Prelu(0.2) + Exp -> ex; DVE builds
    W = mask * ex (one op); PE accumulates [sums | denom] += W.T @ [G|1]
    into the group's 32 PSUM rows (13x4 subtile chain per group).
Counts and the final (sums / denom / counts / a) normalization are
O(segments) and done on host.
"""

from contextlib import ExitStack

import numpy as np

import concourse.bacc as bacc
import concourse.tile as tile
from concourse import mybir
from concourse.bass_utils import run_bass_kernel_spmd

N_CORES = 8
P = 128                 # partitions / nodes per subtile
H = 256                 # hidden
NSEG = 1024
SEG_PER_CORE = NSEG // N_CORES   # 128
K = 4                   # subtiles per supertile
GSEG = 32               # segments per group
NGROUP = SEG_PER_CORE // GSEG    # 4 groups per core
SUP_PER_GROUP = 13      # supertiles per group (6656 nodes >= max group ~6415)
NSUP = NGROUP * SUP_PER_GROUP    # 52 supertiles
GROUP_CAP = SUP_PER_GROUP * K * P   # 6656 nodes per group
NP = NSUP * K * P       # 26624 padded nodes per core
ROW = H + 2 + GSEG      # 290: [256 G | 1.0 | 1 pad | 32 mask]
MASK0 = H + 2           # mask column offset (258 elems = 516B, 4B-aligned)
BATCH = 4               # supertiles per batch
NB = NSUP // BATCH      # 13 batches
C = K * BATCH           # 16 subtiles per batch
CA = 2                  # subtiles per batch reduced on ACT instead of DVE
CD = C - CA             # subtiles per batch reduced on the DVE fold cascade
NEG_SLOPE = 0.2

_G, _OUT = "gfeat", "out"
F32 = mybir.dt.float32
BF16 = mybir.dt.bfloat16
ALU = mybir.AluOpType


def _build_program():
    nc = bacc.Bacc("TRN2", target_bir_lowering=False, debug=False)
    g_d = nc.dram_tensor(_G, [P, NB * C * ROW], BF16, kind="ExternalInput").ap()
    out_d = nc.dram_tensor(_OUT, [P, H + 1], F32, kind="ExternalOutput").ap()
    g_r = g_d.rearrange("p (b c r) -> p b c r", b=NB, c=C)

    with tile.TileContext(nc) as tc, ExitStack() as ctx:
        gpool = ctx.enter_context(tc.tile_pool(name="g", bufs=6))
        fpool = ctx.enter_context(tc.tile_pool(name="f", bufs=2))
        spool = ctx.enter_context(tc.tile_pool(name="s", bufs=1))
        zpool = ctx.enter_context(tc.tile_pool(name="z", bufs=3))
        wpool = ctx.enter_context(tc.tile_pool(name="w", bufs=2))
        opool = ctx.enter_context(tc.tile_pool(name="o", bufs=1))
        psum = ctx.enter_context(tc.tile_pool(name="psum", bufs=1, space="PSUM"))

        acc = psum.tile([P, H + 1], F32, tag="acc")
        ascr = spool.tile([P, H], BF16, tag="ascr")  # ACT accum scratch out

        def z_and_ex(b, Gb):
            """Reduce: DVE fold cascade (CD subtiles) + ACT accum (CA);
            then ACT prelu+exp -> exb."""
            zb = zpool.tile([P, C], F32, name="zb")
            f1 = fpool.tile([P, CD, 128], BF16, name="f1")
            nc.vector.tensor_tensor(out=f1, in0=Gb[:, 0:CD, 0:128],
                                    in1=Gb[:, 0:CD, 128:256], op=ALU.add)
            f2 = fpool.tile([P, CD, 64], BF16, name="f2")
            nc.vector.tensor_tensor(out=f2, in0=f1[:, :, 0:64],
                                    in1=f1[:, :, 64:128], op=ALU.add)
            f3 = fpool.tile([P, CD, 32], BF16, name="f3")
            nc.vector.tensor_tensor(out=f3, in0=f2[:, :, 0:32],
                                    in1=f2[:, :, 32:64], op=ALU.add)
            nc.vector.tensor_reduce(out=zb[:, 0:CD], in_=f3,
                                    axis=mybir.AxisListType.X, op=ALU.add)
            for c in range(CD, C):
                nc.scalar.activation(ascr, Gb[:, c, 0:H],
                                     mybir.ActivationFunctionType.Copy,
                                     accum_out=zb[:, c:c + 1])
            lb = zpool.tile([P, C], F32, name="lb")
            nc.scalar.activation(lb, zb, mybir.ActivationFunctionType.Prelu,
                                 alpha=NEG_SLOPE)
            exb = zpool.tile([P, C], F32, name="exb")
            nc.scalar.activation(exb, lb, mybir.ActivationFunctionType.Exp)
            return exb

        def w_and_matmul(b, Gb, exb):
            W16 = wpool.tile([P, C, GSEG], BF16, name="W16")
            nc.vector.tensor_tensor(
                out=W16, in0=Gb[:, :, MASK0:ROW],
                in1=exb[:, :, None].broadcast_to([P, C, GSEG]),
                op=ALU.mult)
            for c in range(C):
                s = b * BATCH + c // K
                g = s // SUP_PER_GROUP
                j = s % SUP_PER_GROUP
                k = c % K
                nc.tensor.matmul(acc[g * GSEG:(g + 1) * GSEG, :],
                                 lhsT=W16[:, c, :], rhs=Gb[:, c, 0:H + 1],
                                 start=(j == 0 and k == 0),
                                 stop=(j == SUP_PER_GROUP - 1 and k == K - 1),
                                 tile_position=(0, g * GSEG))

        out_sb = opool.tile([P, H + 1], F32, tag="out_sb")

        def emit_group_out(g):
            r0, r1 = g * GSEG, (g + 1) * GSEG
            nc.scalar.copy(out_sb[r0:r1, :], acc[r0:r1, :])
            nc.scalar.dma_start(out_d[r0:r1, :], out_sb[r0:r1, :])

        # group g's accumulation chain closes during batch (13g+12)//4;
        # emit its output 3 batches later so the stop-matmul has retired
        # and the in-order ACT queue never stalls on it.
        out_at = {((SUP_PER_GROUP * (g + 1) - 1) // BATCH) + 3: g
                  for g in range(NGROUP)}

        pending = None          # (b, Gb, exb) awaiting W+matmul
        for b in range(NB):
            Gb = gpool.tile([P, C, ROW], BF16, name="Gb")
            nc.sync.dma_start(Gb, g_r[:, b])
            exb = z_and_ex(b, Gb)
            if pending is not None:
                w_and_matmul(*pending)
            if b in out_at:
                emit_group_out(out_at[b])
            pending = (b, Gb, exb)
        w_and_matmul(*pending)
        emit_group_out(NGROUP - 1)

    nc.compile()
    return nc


def _to_bf16(x):
    return np.asarray(x, dtype=np.float32).astype(mybir.dt.np(BF16))


def kernel(feature, a, batch, _trace=False):
    feature = np.asarray(feature, dtype=np.float32)
    a = np.asarray(a, dtype=np.float32)
    batch = np.asarray(batch)
    n = feature.shape[0]
    assert feature.shape == (n, H) and batch.shape == (n,)

    avec = a.reshape(-1)                      # [256]
    gfull = feature * avec[None, :]           # G = F * a  (fp32, exact mult)

    gbounds = np.searchsorted(batch, np.arange(0, NSEG + 1, GSEG))

    in_maps = []
    for c in range(N_CORES):
        g_c = np.zeros((NP, ROW), dtype=np.float32)
        g_c[:, H] = 1.0                       # denominator ones column
        for g in range(NGROUP):
            gi = c * NGROUP + g
            s, e = int(gbounds[gi]), int(gbounds[gi + 1])
            cnt = e - s
            assert cnt <= GROUP_CAP, (
                f"core {c} group {g} has {cnt} nodes > capacity {GROUP_CAP}")
            base = g * GROUP_CAP
            g_c[base:base + cnt, 0:H] = gfull[s:e]
            seg_rel = batch[s:e].astype(np.int64) - (c * SEG_PER_CORE + g * GSEG)
            g_c[np.arange(base, base + cnt), MASK0 + seg_rel] = 1.0  # one-hot
        # [NP, ROW] -> [NSUP, K, P, ROW] -> [P, (NSUP K ROW)]
        g_t = _to_bf16(
            g_c.reshape(NSUP, K, P, ROW).transpose(2, 0, 1, 3).reshape(P, -1))
        in_maps.append({_G: np.ascontiguousarray(g_t)})

    nc = _build_program()
    res = run_bass_kernel_spmd(nc, in_maps, core_ids=list(range(N_CORES)),
                               trace=_trace)

    counts = np.bincount(batch.astype(np.int64), minlength=NSEG).astype(np.float32)
    counts = np.maximum(counts, 1.0)
    safe_a = np.where(np.abs(avec) > 1e-30, avec, 1e-30)  # [256]
    out = np.zeros((NSEG, H), dtype=np.float32)
    for c in range(N_CORES):
        blk = res.results[c][_OUT]          # [128, 257]
        sums, denom = blk[:, :H], blk[:, H]
        seg0 = c * SEG_PER_CORE
        safe = np.maximum(denom, 1e-30)[:, None]
        out[seg0:seg0 + SEG_PER_CORE] = np.where(
            denom[:, None] > 0.0,
            sums / safe / counts[seg0:seg0 + SEG_PER_CORE, None] / safe_a[None, :],
            0.0,
        )
    if _trace:
        kernel.last_results = res
    return out
